# revision 8
# baseline (speedup 1.0000x reference)
"""Trainium2 Bass kernel for 2D single-level DWT (coif1, symmetric padding).

Input  x: (4, 64, 512, 512) fp32
Output  : (4, 256, 258, 258) fp32  -- per input channel: [cA, cH, cV, cD]

Math: with R_f the banded 258x512 operator of the 1D DWT along an axis
(6-tap filter, stride 2, symmetric boundary folds), the four outputs are
    cA = R_lo X R_lo^T,  cH = R_hi X R_lo^T,
    cV = R_lo X R_hi^T,  cD = R_hi X R_hi^T.

v5 design (per-image pipeline, contiguous 1-bank PSUM chains):
  pass 1 (contract rows r):   Yt_f[c, kh] = sum_r X[r, c] R_f[kh, r]
     stationary lhsT = X chunk [r:128, c:128]; moving rhs = R^T slice with
     the lo/hi filter pair interleaved along the stream dim (col 2*kh+f).
     R is banded: r-chunk q only reaches khf cols [128q, 128q+132).
  pass 2 (contract cols c):   O[kh, kwg] = sum_c Yt_f[c, kh] R_g[kw, c]
     stationary lhsT = stride-2 slice of Yt (kh chunk for filter f);
     moving rhs = the SAME banded weight tensor.
  Each chain writes khf/kwg cols [0,512) into ONE psum bank and the
  4-col tail [512,516) into a slot of a shared tail bank, so the main
  drain is a single contiguous [128,512] fp32->fp16 copy (strided drains
  cost ~2x on ACT/DVE).  Tail slots are drained once per image with one
  small strided copy.  Drains are split across scalar+vector by time.
  The PE runs pass1(i+1) before pass2(i) so drains trail a full image
  behind the producer.  DMA: per-image transfers (input 2 halves on the
  sync queue, output on gpsimd), input prefetch depth 5.
  A post-legalize pass drops LDWEIGHTS whose weights AP equals the
  previous load on the final PE stream (the PE keeps its stationary
  across matmuls), saving ~105ns of queue occupancy each.
"""

import os
import sys

for _p in ("/opt/trn_rl_repo", "/opt/pypackages"):
    if _p not in sys.path:
        sys.path.append(_p)

os.environ.setdefault("JAX_COMPILATION_CACHE_DIR", "/tmp/jax_comp_cache")
os.environ.setdefault("JAX_PERSISTENT_CACHE_MIN_COMPILE_TIME_SECS", "10")

import numpy as np

import concourse.bass as bass
import concourse.bacc as bacc
import concourse.mybir as mybir
import concourse.tile as _tile_mod
from concourse.bass_utils import run_bass_kernel_spmd
from concourse.tile import TileContext

N_CORES = 8
H = W = 512
OUT = 258  # (512 + 6 - 1) // 2
IMGS = 32  # images per core (4*64/8)
F16 = mybir.dt.float16
F32 = mybir.dt.float32

# pywt coif1 decomposition filters
DEC_LO = np.array([-0.01565572813546454, -0.0727326195128539, 0.38486484686420286,
                   0.8525720202122554, 0.3378976624578092, -0.0727326195128539])
DEC_HI = np.array([0.0727326195128539, 0.3378976624578092, -0.8525720202122554,
                   0.38486484686420286, 0.0727326195128539, -0.01565572813546454])
FLEN = 6
PAD = 4
LO_F = DEC_LO[::-1]
HI_F = DEC_HI[::-1]

# kh/kw window that r/c-chunk q contributes to (from the band structure)
WINS = [(0, 66), (64, 130), (128, 194), (192, 258)]


def _dedup_ldweights(ordered, nc):
    """Drop InstLdweights whose weights AP is identical to the previous
    PE weight load in the final post-schedule order (only matmuls in
    between).  The PE array keeps its stationary operand across matmuls,
    so the reload is a no-op; removal is done only when the candidate's
    dep edges are covered by the live load and nothing depends on it."""
    PE = mybir.EngineType.PE
    for bb, insts in ordered.items():
        cur_sig = None
        cur_deps = None
        keep = []
        for inst in insts:
            if getattr(inst, "engine", None) != PE:
                keep.append(inst)
                continue
            tn = type(inst).__name__
            if tn == "InstLdweights":
                sig = str(inst.ins[0])
                deps = (frozenset(inst.sync_dependency_names()),
                        frozenset(inst.nosync_dependency_names()))
                if (cur_sig is not None and sig == cur_sig
                        and deps[0] <= cur_deps[0] and deps[1] <= cur_deps[1]
                        and not inst.descendants):
                    continue
                cur_sig, cur_deps = sig, deps
                keep.append(inst)
            elif tn == "InstMatmult":
                if inst.is_transpose:
                    cur_sig = None
                keep.append(inst)
            else:
                keep.append(inst)
        ordered[bb] = keep
    return ordered


_orig_tile_legalize = _tile_mod.tile_legalize


def _legalize_with_dedup(ordered, nc):
    return _dedup_ldweights(_orig_tile_legalize(ordered, nc), nc)


_tile_mod.tile_legalize = _legalize_with_dedup


def _build_R(filt: np.ndarray, n: int = W) -> np.ndarray:
    """Banded [258, 512] operator: out[k] = sum_j filt[j] * x[sym(2k + j - PAD)]."""
    out_len = (n + FLEN - 1) // 2

    def sym(i: int) -> int:
        while i < 0 or i >= n:
            if i < 0:
                i = -i - 1
            if i >= n:
                i = 2 * n - 1 - i
        return i

    R = np.zeros((out_len, n), dtype=np.float64)
    for k in range(out_len):
        for j in range(FLEN):
            R[k, sym(2 * k + j - PAD)] += filt[j]
    return R


def _build_weights() -> np.ndarray:
    """Interleaved: w[p, q*516 + 2k + f] = R_f[k, 128q + p], [128, 4*516] fp16."""
    Rs = [_build_R(LO_F), _build_R(HI_F)]
    w = np.zeros((128, 4 * 2 * OUT), dtype=np.float32)
    for q in range(4):
        blk = np.zeros((128, OUT, 2), dtype=np.float32)
        for f in range(2):
            blk[:, :, f] = Rs[f][:, 128 * q:128 * (q + 1)].T
        w[:, q * 2 * OUT:(q + 1) * 2 * OUT] = blk.reshape(128, 2 * OUT)
    return w.astype(np.float16)


_WEIGHTS = _build_weights()
_MODULE = None

# chain segments: (q, psum col lo, psum col hi) over the khf/kwg index
# [0,516); q's band is [128q, 128q+132), clipped at the 512 bank edge.
# The [512,516) remainder goes to a tail-bank slot.
SEGS_MAIN = [(0, 0, 132), (1, 128, 260), (2, 256, 388), (3, 384, 512)]
SEG_TAIL = (3, 512, 516)


def _wslice(Wr, q, lo, hi):
    base = q * 516 + 128 * q
    return Wr[:, base + (lo - 128 * q):base + (hi - 128 * q)]


def _build_module() -> bass.Bass:
    nc = bacc.Bacc("TRN2", target_bir_lowering=False, debug=False)
    x_in = nc.declare_dram_parameter("x", [IMGS, 128, 4 * W], F16, isOutput=False)
    w_in = nc.declare_dram_parameter("w", [128, 4 * 516], F16, isOutput=False)
    # y[i, p, (2*khc+f)*516 + 2*kw + g] = O_{f+2g}[128*khc + p, kw]
    y_out = nc.declare_dram_parameter("y", [IMGS, 128, 4 * 516], F16,
                                      isOutput=True)
    # yr[2*j + f, i*516 + 2*kw + g] = O_{f+2g}[256 + j, kw]
    yr_out = nc.declare_dram_parameter("yr", [4, IMGS * 516], F16,
                                       isOutput=True)

    PREF = 5  # input prefetch depth (images)

    with TileContext(nc) as tc:
        with (
            tc.tile_pool(name="wpool", bufs=1) as wpool,
            tc.tile_pool(name="xpool", bufs=PREF) as xpool,
            tc.tile_pool(name="ypool", bufs=3) as ypool,
            tc.tile_pool(name="spool", bufs=3) as spool,
            tc.tile_pool(name="rpool", bufs=1) as rpool,
            tc.tile_pool(name="p1", bufs=4, space="PSUM") as p1pool,
            tc.tile_pool(name="p2", bufs=3, space="PSUM") as p2pool,
            tc.tile_pool(name="pt", bufs=1, space="PSUM") as ptpool,
        ):
            Wt = wpool.tile([128, 4 * 516], F16)
            Crem = rpool.tile([4, IMGS * 516], F16)
            # shared tail bank: slots 0-3 pass1(cc), 4-7 pass2(blk), 8 rem
            Tb = ptpool.tile([128, 36], F32)

            def load_x(i):
                X = xpool.tile([128, 4 * W], F16, tag="X", name=f"X_{i}")
                # two halves so compute can start on q0/q1 early
                nc.sync.dma_start(out=X[:, 0:2 * W], in_=x_in[i, :, 0:2 * W])
                nc.sync.dma_start(out=X[:, 2 * W:4 * W],
                                  in_=x_in[i, :, 2 * W:4 * W])
                return X

            Xg = {i: load_x(i) for i in range(min(PREF - 1, IMGS))}
            nc.gpsimd.dma_start(out=Wt[:], in_=w_in[:])
            Wr = Wt[:]

            # Tiny PE op consuming the weight DMA so later matmuls depend
            # on it via PE program order.
            warm = p1pool.tile([128, 512], F32, tag="p1")
            nc.tensor.matmul(warm[0:1, 0:1], lhsT=Wr[:, 0:1], rhs=Wr[:, 0:1],
                             start=True, stop=True)

            def copy(dst, src, eng):
                if eng == "s":
                    nc.scalar.copy(out=dst, in_=src)
                else:
                    nc.vector.tensor_copy(out=dst, in_=src)

            def chain(lhsT_fn, M, np_, ts):
                """Banded filter-interleaved accumulation chain: cols
                [0,512) accumulate in the 1-bank tile M; the [512,516)
                remainder is a fresh-write matmul into slot ts of the
                shared tail bank (drained separately, once per image)."""
                n = len(SEGS_MAIN)
                for si, (q, lo, hi) in enumerate(SEGS_MAIN):
                    nc.tensor.matmul(M[:, lo:hi], lhsT=lhsT_fn(q),
                                     rhs=_wslice(Wr, q, lo, hi),
                                     start=(si == 0), stop=(si == n - 1))
                q, lo, hi = SEG_TAIL
                nc.tensor.matmul(Tb[0:np_, 4 * ts:4 * ts + 4],
                                 lhsT=lhsT_fn(q), rhs=_wslice(Wr, q, lo, hi),
                                 start=True, stop=True)

            def pass1(i, Yt):
                Xv = Xg[i][:]
                for cc in range(4):
                    M = p1pool.tile([128, 512], F32, tag="p1")
                    chain(lambda q: Xv[:, q * W + cc * 128:q * W + (cc + 1) * 128],
                          M[:], 128, cc)
                    # contiguous [128,512] drain; alternate engines
                    copy(Yt[:, cc * 516:cc * 516 + 512], M[:],
                         "v" if cc % 2 == 0 else "s")
                # tail slots 0..3 -> Yt cols {cc*516 + 512..516}
                Ytv = Yt[:].rearrange("p (cc k) -> p cc k", cc=4)
                copy(Ytv[:, :, 512:516], Tb[:, 0:16].rearrange(
                    "p (cc k) -> p cc k", cc=4), "v")

            def pass2(i, Yt, STG):
                # col = cc*516 + 2*k + f  (k in [0,258), f interleaved)
                Ytv = Yt[:].rearrange("p (cc k f) -> p cc k f", cc=4, f=2)
                for blk in range(4):
                    khc, f = blk // 2, blk % 2
                    M = p2pool.tile([128, 512], F32, tag="p2")
                    chain(lambda q: Ytv[:, q, 128 * khc:128 * (khc + 1), f],
                          M[:], 128, 4 + blk)
                    copy(STG[:, blk * 516:blk * 516 + 512], M[:],
                         "v" if blk % 2 == 0 else "s")
                Sv = STG[:].rearrange("p (blk k) -> p blk k", blk=4)
                copy(Sv[:, :, 512:516], Tb[:, 16:32].rearrange(
                    "p (blk k) -> p blk k", blk=4), "v")

            def rem(i, Yt):
                # kh in {256,257}: lhsT = the 4 tail cols of each Yt block
                Ytv = Yt[:].rearrange("p (cc k) -> p cc k", cc=4)
                M = p1pool.tile([128, 512], F32, tag="p1")
                chain(lambda q: Ytv[:, q, 512:516], M[0:4, :], 4, 8)
                copy(Crem[:, i * 516:i * 516 + 512], M[0:4, :], "s")
                copy(Crem[:, i * 516 + 512:(i + 1) * 516], Tb[0:4, 32:36], "s")

            # software pipeline: PE runs pass1(i+1) before pass2(i)
            Ytg = {0: ypool.tile([128, 4 * 516], F16, tag="Yt", name="Yt_0")}
            pass1(0, Ytg[0])
            for i in range(IMGS):
                if i + PREF - 1 < IMGS:
                    Xg[i + PREF - 1] = load_x(i + PREF - 1)
                if i + 1 < IMGS:
                    Ytg[i + 1] = ypool.tile([128, 4 * 516], F16, tag="Yt",
                                            name=f"Yt_{i + 1}")
                    pass1(i + 1, Ytg[i + 1])
                STG = spool.tile([128, 4 * 516], F16, tag="STG")
                pass2(i, Ytg[i], STG)
                rem(i, Ytg[i])
                del Ytg[i]
                ring = nc.sync if i == IMGS - 1 else nc.gpsimd
                ring.dma_start(out=y_out[i], in_=STG[:])
                if i % 8 == 7:
                    c = i // 8
                    nc.gpsimd.dma_start(
                        out=yr_out[:, c * 8 * 516:(c + 1) * 8 * 516],
                        in_=Crem[:, c * 8 * 516:(c + 1) * 8 * 516])
    nc.finalize()
    return nc


def _get_module() -> bass.Bass:
    global _MODULE
    if _MODULE is None:
        _MODULE = _build_module()
    return _MODULE


def make_in_maps(x: np.ndarray) -> list[dict]:
    x = np.asarray(x, dtype=np.float32)
    B, C, Hx, Wx = x.shape
    assert (Hx, Wx) == (H, W) and B * C == N_CORES * IMGS
    imgs = x.reshape(B * C, H, W)
    maps = []
    for k in range(N_CORES):
        # X[i][p, q*512 + c] = x[i, 128q + p, c]
        xc = imgs[k * IMGS:(k + 1) * IMGS].reshape(IMGS, 4, 128, W)
        xc = np.ascontiguousarray(xc.transpose(0, 2, 1, 3))
        maps.append({"x": xc.reshape(IMGS, 128, 4 * W).astype(np.float16),
                     "w": _WEIGHTS})
    return maps


def kernel(**inputs) -> np.ndarray:
    x = np.asarray(inputs["x"], dtype=np.float32)
    B, C, Hx, Wx = x.shape

    nc = _get_module()
    in_maps = make_in_maps(x)
    res = run_bass_kernel_spmd(nc, in_maps, list(range(N_CORES))).results

    full = np.empty((N_CORES * IMGS, 4, OUT, OUT), dtype=np.float32)
    for k in range(N_CORES):
        # ym[i, p, khc, f, kw, g] = O_{f+2g}[128*khc+p, kw]
        ym = res[k]["y"].reshape(IMGS, 128, 2, 2, OUT, 2)
        # yr[2*j + f, i, kw, g] = O_{f+2g}[256+j, kw]
        yr = res[k]["yr"].reshape(2, 2, IMGS, OUT, 2)
        dst = full[k * IMGS:(k + 1) * IMGS]
        # dst[i, 2g+f, khc*128+p, kw]
        t = ym.transpose(0, 5, 3, 2, 1, 4).reshape(IMGS, 4, 256, OUT)
        dst[:, :, :256, :] = t
        # remainder rows: yr[j, f, i, kw, g] -> dst[i, 2g+f, 256+j, kw]
        r = yr.transpose(2, 4, 1, 0, 3)  # [i, g, f, j, kw]
        dst[:, :, 256:258, :] = r.reshape(IMGS, 4, 2, OUT)

    return np.ascontiguousarray(full.reshape(B, 4 * C, OUT, OUT))


# revision 9
# speedup vs baseline: 1.0555x; 1.0555x over previous
"""Trainium2 Bass kernel for 2D single-level DWT (coif1, symmetric padding).

Input  x: (4, 64, 512, 512) fp32
Output  : (4, 256, 258, 258) fp32  -- per input channel: [cA, cH, cV, cD]

Math: with R_f the banded 258x512 operator of the 1D DWT along an axis
(6-tap filter, stride 2, symmetric boundary folds), the four outputs are
    cA = R_lo X R_lo^T,  cH = R_hi X R_lo^T,
    cV = R_lo X R_hi^T,  cD = R_hi X R_hi^T.

v4 design (fp16 data path, band-windowed matmuls, 32 images per core):
  pass 1 (contract rows r):   Yt_f[c, kh] = sum_r X[r, c] R_f[kh, r]
     stationary lhsT = X chunk [r:128, c:128]; moving rhs = R^T slice with
     the lo/hi filter pair interleaved along the stream dim (col 2*kh+f),
     so one matmul serves both filters per LDWEIGHTS.  R is banded:
     r-chunk q only reaches kh in [64q, 64q+66), so each matmul streams
     ~132 interleaved columns instead of 516.
  pass 2 (contract cols c):   O_s[kh, kw] = sum_c Yt_f[c, kh] R_g[kw, c]
     stationary lhsT = Yt chunk (stride-2 slice of the interleaved Yt);
     kh tiled [0,128), [128,256), plus a 2-row remainder whose lhsT is the
     4 contiguous tail columns of each Yt block.
  PSUM accumulation relies on per-element has_written bits: first matmul
  into a bank uses start=True (arms lazy-zero for the whole bank); later
  chain matmuls use start=False and may touch a mix of written
  (accumulate) and pending-zero (overwrite) columns.
  Chains are PAIRED into double-width PSUM tiles (A-pair spans 2 banks,
  B-pair shares 1 bank) so one engine copy drains two chains -- the
  scalar engine pays ~200ns fixed cost per instruction, so fewer, bigger
  drains matter.  The PE runs pass1(i+1) before pass2(i) so drains always
  trail a full chain-group behind the producer (no PSUM-recycle stalls).
  DMA: 2-image granularity, 128 partitions x >=2KiB contiguous per
  partition per transfer (spreads over all 16 SDMA engines).
"""

import os
import sys

for _p in ("/opt/trn_rl_repo", "/opt/pypackages"):
    if _p not in sys.path:
        sys.path.append(_p)

os.environ.setdefault("JAX_COMPILATION_CACHE_DIR", "/tmp/jax_comp_cache")
os.environ.setdefault("JAX_PERSISTENT_CACHE_MIN_COMPILE_TIME_SECS", "10")

import numpy as np

import concourse.bass as bass
import concourse.bacc as bacc
import concourse.mybir as mybir
from concourse.bass_utils import run_bass_kernel_spmd
from concourse.tile import TileContext

N_CORES = 8
H = W = 512
OUT = 258  # (512 + 6 - 1) // 2
IMGS = 32  # images per core (4*64/8)
GRP = 2    # images per DMA transfer
NG = IMGS // GRP
F16 = mybir.dt.float16
F32 = mybir.dt.float32

# pywt coif1 decomposition filters
DEC_LO = np.array([-0.01565572813546454, -0.0727326195128539, 0.38486484686420286,
                   0.8525720202122554, 0.3378976624578092, -0.0727326195128539])
DEC_HI = np.array([0.0727326195128539, 0.3378976624578092, -0.8525720202122554,
                   0.38486484686420286, 0.0727326195128539, -0.01565572813546454])
FLEN = 6
PAD = 4
LO_F = DEC_LO[::-1]
HI_F = DEC_HI[::-1]

# kh/kw window that r/c-chunk q contributes to (from the band structure)
WINS = [(0, 66), (64, 130), (128, 194), (192, 258)]
BSP = 194  # per-chain PSUM split: [0,194)x2 = 1552B (A), [194,258)x2 = 512B (B)

# If True, split matmuls so no instruction touches a mix of
# already-written and pending-zero PSUM bytes (needed only for CoreSim;
# hardware has per-element has_written bits).
INTERP_SAFE = False


def _build_R(filt: np.ndarray, n: int = W) -> np.ndarray:
    """Banded [258, 512] operator: out[k] = sum_j filt[j] * x[sym(2k + j - PAD)]."""
    out_len = (n + FLEN - 1) // 2

    def sym(i: int) -> int:
        while i < 0 or i >= n:
            if i < 0:
                i = -i - 1
            if i >= n:
                i = 2 * n - 1 - i
        return i

    R = np.zeros((out_len, n), dtype=np.float64)
    for k in range(out_len):
        for j in range(FLEN):
            R[k, sym(2 * k + j - PAD)] += filt[j]
    return R


def _check_windows(R: np.ndarray) -> None:
    for q in range(4):
        nz = np.nonzero((R[:, 128 * q:128 * (q + 1)] != 0).any(axis=1))[0]
        assert (int(nz.min()), int(nz.max()) + 1) == WINS[q], (q, nz.min(), nz.max())


def _build_weights() -> np.ndarray:
    """Interleaved: w[p, q*516 + 2k + f] = R_f[k, 128q + p], [128, 4*516] fp16."""
    Rs = [_build_R(LO_F), _build_R(HI_F)]
    _check_windows(Rs[0])
    _check_windows(Rs[1])
    w = np.zeros((128, 4 * 2 * OUT), dtype=np.float32)
    for q in range(4):
        blk = np.zeros((128, OUT, 2), dtype=np.float32)
        for f in range(2):
            blk[:, :, f] = Rs[f][:, 128 * q:128 * (q + 1)].T
        w[:, q * 2 * OUT:(q + 1) * 2 * OUT] = blk.reshape(128, 2 * OUT)
    return w.astype(np.float16)


_WEIGHTS = _build_weights()
_MODULE = None


def _build_module() -> bass.Bass:
    nc = bacc.Bacc("TRN2", target_bir_lowering=False, debug=False)
    x_in = nc.declare_dram_parameter("x", [NG, 128, GRP * 4 * W], F16,
                                     isOutput=False)
    w_in = nc.declare_dram_parameter("w", [128, 4 * 2 * OUT], F16, isOutput=False)
    # y[g, p, ((ig*2 + khc)*2 + f)*516 + 2*kw + gg] = O_{f+2gg}[128*khc + p, kw]
    y_main = nc.declare_dram_parameter("y", [NG, 128, GRP * 4 * 516], F16,
                                       isOutput=True)
    # yr[j*4 + f*2 + ig, g*516 + 2*kw + gg] = O_{f+2gg}[256 + j, kw]
    y_rem = nc.declare_dram_parameter("yr", [8, NG * 516], F16,
                                     isOutput=True)

    with TileContext(nc) as tc:
        with (
            tc.tile_pool(name="wpool", bufs=1) as wpool,
            tc.tile_pool(name="xpool", bufs=3) as xpool,
            tc.tile_pool(name="ypool", bufs=2) as ypool,
            tc.tile_pool(name="spool", bufs=2) as spool,
            tc.tile_pool(name="rpool", bufs=1) as rpool,
            tc.tile_pool(name="psum", bufs=2, space="PSUM") as pspool,
        ):
            Wt = wpool.tile([128, 4 * 2 * OUT], F16)
            Crem = rpool.tile([8, NG * 516], F16)

            def load_x(g):
                X = xpool.tile([128, GRP * 4 * W], F16, tag="X", name=f"X_{g}")
                nc.sync.dma_start(out=X[:, 0:4 * W], in_=x_in[g, :, 0:4 * W])
                nc.sync.dma_start(out=X[:, 4 * W:8 * W],
                                  in_=x_in[g, :, 4 * W:8 * W])
                return X

            X0 = load_x(0)
            nc.gpsimd.dma_start(out=Wt[:], in_=w_in[:])
            Wr = Wt[:]

            # Tiny PE op consuming the weight DMA so later matmuls depend
            # on it via PE program order.
            warm = pspool.tile([1, 256], F32, tag="pBB", bufs=2)
            nc.tensor.matmul(warm[:, 0:1], lhsT=Wr[:, 0:1], rhs=Wr[:, 0:1],
                             start=True, stop=True)

            def copy(dst, src, eng):
                if eng == "s":
                    nc.scalar.copy(out=dst, in_=src)
                else:
                    nc.vector.tensor_copy(out=dst, in_=src)

            def chain(lhsT_fn, A, B, ha, hb):
                """One banded, filter-interleaved accumulation chain into
                half `ha` of A-pair tile A (512-elem halves = bank-aligned)
                and half `hb` of B-pair tile B (128-elem halves)."""
                if INTERP_SAFE:
                    segs = [(0, 0, 66, 0, True, False),
                            (1, 64, 66, 0, False, False),
                            (1, 66, 130, 0, False, False),
                            (2, 128, 130, 0, False, False),
                            (2, 130, 194, 0, False, False),
                            (3, 192, 194, 0, False, True),
                            (3, 194, 258, 1, True, True)]
                else:
                    segs = [(0, 0, 66, 0, True, False),
                            (1, 64, 130, 0, False, False),
                            (2, 128, 194, 0, False, False),
                            (3, 192, 194, 0, False, True),
                            (3, 194, 258, 1, True, True)]
                for q, lo, hi, t, st, sp in segs:
                    if t == 0:
                        out = A[:, ha * 512 + 2 * lo:ha * 512 + 2 * hi]
                    else:
                        out = B[:, hb * 128 + 2 * (lo - BSP):
                                hb * 128 + 2 * (hi - BSP)]
                    rhs = Wr[:, q * 2 * OUT + 2 * lo:q * 2 * OUT + 2 * hi]
                    nc.tensor.matmul(out, lhsT=lhsT_fn(q), rhs=rhs,
                                     start=st, stop=sp)

            def pair_views(A, B):
                Ah = A[:].rearrange("p (h k) -> p h k", h=2)[:, :, 0:2 * BSP]
                Bh = B[:].rearrange("p (h k) -> p h k", h=2)
                return Ah, Bh

            def pass1(Xv, Yt, ig):
                """4 paired chains into the group Yt tile:
                Yt[p, cc*1032 + (2*kh + f)*2 + ig]."""
                Ytv = Yt[:].rearrange("p (cc k i) -> p cc k i", cc=4, i=GRP)
                for cp in range(2):  # cc pairs (0,1), (2,3)
                    A = pspool.tile([128, 1024], F32, tag="pAA", bufs=3)
                    B = pspool.tile([128, 256], F32, tag="pBB", bufs=2)
                    for h in range(2):
                        cc = cp * 2 + h
                        chain(lambda q: Xv[:, ig, q, cc * 128:(cc + 1) * 128],
                              A[:], B[:], h, h)
                    Ah, Bh = pair_views(A, B)
                    copy(Ytv[:, 2 * cp:2 * cp + 2, 0:2 * BSP, ig], Ah,
                         "s" if cp == 0 else "v")
                    copy(Ytv[:, 2 * cp:2 * cp + 2, 2 * BSP:516, ig], Bh,
                         "v" if cp == 0 else "s")

            def pass2(Yt, STG, ig, i):
                Ytr = Yt[:]
                Ytv4 = Ytr.rearrange("p (cc k f i) -> p cc k f i",
                                     cc=4, f=2, i=GRP)
                Sv = STG[:].rearrange("p (blk k) -> p blk k", k=516)
                for khc in range(2):  # pair over f
                    A = pspool.tile([128, 1024], F32, tag="pAA", bufs=3)
                    B = pspool.tile([128, 256], F32, tag="pBB", bufs=2)
                    for f in range(2):
                        chain(lambda q: Ytv4[:, q,
                                             128 * khc:128 * (khc + 1), f, ig],
                              A[:], B[:], f, f)
                    Ah, Bh = pair_views(A, B)
                    base = (ig * 2 + khc) * 2
                    copy(Sv[:, base:base + 2, 0:2 * BSP], Ah,
                         "s" if khc == 0 else "v")
                    copy(Sv[:, base:base + 2, 2 * BSP:516], Bh, "v")

            def rem(Yt, g):
                # remainder rows kh in {256,257} for the whole group: lhsT =
                # 8 contiguous tail cols of each Yt block (order (j, f, ig));
                # psum rows j*4 + f*2 + ig.  One pAA tile: A part in bank 0,
                # B part at the start of bank 1.
                Ytr = Yt[:]
                Rt = pspool.tile([8, 1024], F32, tag="pAA", bufs=3)
                chain(lambda q: Ytr[:, q * 1032 + 1024:(q + 1) * 1032],
                      Rt[:], Rt[:], 0, 4)
                copy(Crem[:, g * 516:g * 516 + 2 * BSP], Rt[:, 0:2 * BSP], "v")
                copy(Crem[:, g * 516 + 2 * BSP:(g + 1) * 516],
                     Rt[:, 512:640], "s")

            # software pipeline: PE runs pass1(i+1) before pass2(i)
            Xg = {0: X0, 1: load_x(1)}
            Xv = {g: Xg[g][:].rearrange("p (i q c) -> p i q c", i=GRP, q=4)
                  for g in (0, 1)}
            Ytg = {0: ypool.tile([128, 4 * 516 * GRP], F16, tag="Yt",
                     name="Yt_0")}
            pass1(Xv[0], Ytg[0], 0)
            STG = None
            for i in range(IMGS):
                g, ig = divmod(i, GRP)
                if ig == 0:
                    if g + 2 < NG:
                        Xg[g + 2] = load_x(g + 2)
                        Xv[g + 2] = Xg[g + 2][:].rearrange(
                            "p (i q c) -> p i q c", i=GRP, q=4)
                    STG = spool.tile([128, GRP * 4 * 516], F16, tag="STG")
                if i + 1 < IMGS:
                    g1, ig1 = divmod(i + 1, GRP)
                    if g1 not in Ytg:
                        Ytg[g1] = ypool.tile([128, 4 * 516 * GRP], F16,
                                             tag="Yt", name=f"Yt_{g1}")
                    pass1(Xv[g1], Ytg[g1], ig1)
                pass2(Ytg[g], STG, ig, i)
                if ig == GRP - 1:
                    rem(Ytg[g], g)
                    del Ytg[g]
                    ring = nc.sync if g == NG - 1 else nc.gpsimd
                    ring.dma_start(out=y_main[g], in_=STG[:])
                if i % 8 == 7:
                    c = i // 8
                    nc.gpsimd.dma_start(
                        out=y_rem[:, c * 4 * 516:(c + 1) * 4 * 516],
                        in_=Crem[:, c * 4 * 516:(c + 1) * 4 * 516])
    nc.finalize()
    return nc


def _get_module() -> bass.Bass:
    global _MODULE
    if _MODULE is None:
        _MODULE = _build_module()
    return _MODULE


def make_in_maps(x: np.ndarray) -> list[dict]:
    x = np.asarray(x, dtype=np.float32)
    B, C, Hx, Wx = x.shape
    assert (Hx, Wx) == (H, W) and B * C == N_CORES * IMGS
    imgs = x.reshape(B * C, H, W)
    maps = []
    for k in range(N_CORES):
        # X[g][p, ig*2048 + q*512 + c] = x[g*GRP+ig, 128q + p, c]
        xc = imgs[k * IMGS:(k + 1) * IMGS].reshape(NG, GRP, 4, 128, W)
        xc = np.ascontiguousarray(xc.transpose(0, 3, 1, 2, 4))
        maps.append({"x": xc.reshape(NG, 128, GRP * 4 * W).astype(np.float16),
                     "w": _WEIGHTS})
    return maps


def kernel(**inputs) -> np.ndarray:
    x = np.asarray(inputs["x"], dtype=np.float32)
    B, C, Hx, Wx = x.shape

    nc = _get_module()
    in_maps = make_in_maps(x)
    res = run_bass_kernel_spmd(nc, in_maps, list(range(N_CORES))).results

    full = np.empty((N_CORES * IMGS, 4, OUT, OUT), dtype=np.float32)
    for k in range(N_CORES):
        # [g, p, ig, khc, f, kw, gg]
        ym = res[k]["y"].reshape(NG, 128, GRP, 2, 2, OUT, 2)
        yr = res[k]["yr"].reshape(8, NG, OUT, 2)  # [j*4+f*2+ig, g, kw, gg]
        dst = full[k * IMGS:(k + 1) * IMGS]
        # dst[g*GRP+ig, f+2gg, khc*128+p, kw] = ym[g, p, ig, khc, f, kw, gg]
        t = ym.transpose(0, 2, 4, 6, 3, 1, 5).reshape(IMGS, 4, 256, OUT)
        # t's dim-1 is f*2+gg; reorder to s = f+2gg -> fg indices [0,2,1,3]
        dst[:, :, :256, :] = t[:, [0, 2, 1, 3]]
        for f in range(2):
            for j in range(2):
                for g in range(2):
                    for ig in range(GRP):
                        dst[ig::GRP, f + 2 * g, 256 + j, :] = \
                            yr[j * 4 + f * 2 + ig, :, :, g]

    return np.ascontiguousarray(full.reshape(B, 4 * C, OUT, OUT))



# revision 10
# speedup vs baseline: 1.1013x; 1.0434x over previous
"""Trainium2 Bass kernel for 2D single-level DWT (coif1, symmetric padding).

Input  x: (4, 64, 512, 512) fp32
Output  : (4, 256, 258, 258) fp32  -- per input channel: [cA, cH, cV, cD]

Math: with R_f the banded 258x512 operator of the 1D DWT along an axis
(6-tap filter, stride 2, symmetric boundary folds), the four outputs are
    cA = R_lo X R_lo^T,  cH = R_hi X R_lo^T,
    cV = R_lo X R_hi^T,  cD = R_hi X R_hi^T.

v4 design (fp16 data path, band-windowed matmuls, 32 images per core):
  pass 1 (contract rows r):   Yt_f[c, kh] = sum_r X[r, c] R_f[kh, r]
     stationary lhsT = X chunk [r:128, c:128]; moving rhs = R^T slice with
     the lo/hi filter pair interleaved along the stream dim (col 2*kh+f),
     so one matmul serves both filters per LDWEIGHTS.  R is banded:
     r-chunk q only reaches kh in [64q, 64q+66), so each matmul streams
     ~132 interleaved columns instead of 516.
  pass 2 (contract cols c):   O_s[kh, kw] = sum_c Yt_f[c, kh] R_g[kw, c]
     stationary lhsT = Yt chunk (stride-2 slice of the interleaved Yt);
     kh tiled [0,128), [128,256), plus a 2-row remainder whose lhsT is the
     4 contiguous tail columns of each Yt block.
  PSUM accumulation relies on per-element has_written bits: first matmul
  into a bank uses start=True (arms lazy-zero for the whole bank); later
  chain matmuls use start=False and may touch a mix of written
  (accumulate) and pending-zero (overwrite) columns.
  Chains are PAIRED into double-width PSUM tiles (A-pair spans 2 banks,
  B-pair shares 1 bank) so one engine copy drains two chains -- the
  scalar engine pays ~200ns fixed cost per instruction, so fewer, bigger
  drains matter.  The PE runs pass1(i+1) before pass2(i) so drains always
  trail a full chain-group behind the producer (no PSUM-recycle stalls).
  DMA: 2-image granularity, 128 partitions x >=2KiB contiguous per
  partition per transfer (spreads over all 16 SDMA engines).
"""

import os
import sys

for _p in ("/opt/trn_rl_repo", "/opt/pypackages"):
    if _p not in sys.path:
        sys.path.append(_p)

os.environ.setdefault("JAX_COMPILATION_CACHE_DIR", "/tmp/jax_comp_cache")
os.environ.setdefault("JAX_PERSISTENT_CACHE_MIN_COMPILE_TIME_SECS", "10")

import numpy as np

import concourse.bass as bass
import concourse.bacc as bacc
import concourse.mybir as mybir
from concourse.bass_utils import run_bass_kernel_spmd
from concourse.tile import TileContext

N_CORES = 8
H = W = 512
OUT = 258  # (512 + 6 - 1) // 2
IMGS = 32  # images per core (4*64/8)
GRP = 2    # images per DMA transfer
NG = IMGS // GRP
F16 = mybir.dt.float16
F32 = mybir.dt.float32

# pywt coif1 decomposition filters
DEC_LO = np.array([-0.01565572813546454, -0.0727326195128539, 0.38486484686420286,
                   0.8525720202122554, 0.3378976624578092, -0.0727326195128539])
DEC_HI = np.array([0.0727326195128539, 0.3378976624578092, -0.8525720202122554,
                   0.38486484686420286, 0.0727326195128539, -0.01565572813546454])
FLEN = 6
PAD = 4
LO_F = DEC_LO[::-1]
HI_F = DEC_HI[::-1]

# kh/kw window that r/c-chunk q contributes to (from the band structure)
WINS = [(0, 66), (64, 130), (128, 194), (192, 258)]
BSP = 194  # per-chain PSUM split: [0,194)x2 = 1552B (A), [194,258)x2 = 512B (B)

# If True, split matmuls so no instruction touches a mix of
# already-written and pending-zero PSUM bytes (needed only for CoreSim;
# hardware has per-element has_written bits).
INTERP_SAFE = False


def _build_R(filt: np.ndarray, n: int = W) -> np.ndarray:
    """Banded [258, 512] operator: out[k] = sum_j filt[j] * x[sym(2k + j - PAD)]."""
    out_len = (n + FLEN - 1) // 2

    def sym(i: int) -> int:
        while i < 0 or i >= n:
            if i < 0:
                i = -i - 1
            if i >= n:
                i = 2 * n - 1 - i
        return i

    R = np.zeros((out_len, n), dtype=np.float64)
    for k in range(out_len):
        for j in range(FLEN):
            R[k, sym(2 * k + j - PAD)] += filt[j]
    return R


def _check_windows(R: np.ndarray) -> None:
    for q in range(4):
        nz = np.nonzero((R[:, 128 * q:128 * (q + 1)] != 0).any(axis=1))[0]
        assert (int(nz.min()), int(nz.max()) + 1) == WINS[q], (q, nz.min(), nz.max())


def _build_weights() -> np.ndarray:
    """Interleaved: w[p, q*516 + 2k + f] = R_f[k, 128q + p], [128, 4*516] fp16."""
    Rs = [_build_R(LO_F), _build_R(HI_F)]
    _check_windows(Rs[0])
    _check_windows(Rs[1])
    w = np.zeros((128, 4 * 2 * OUT), dtype=np.float32)
    for q in range(4):
        blk = np.zeros((128, OUT, 2), dtype=np.float32)
        for f in range(2):
            blk[:, :, f] = Rs[f][:, 128 * q:128 * (q + 1)].T
        w[:, q * 2 * OUT:(q + 1) * 2 * OUT] = blk.reshape(128, 2 * OUT)
    return w.astype(np.float16)


_WEIGHTS = _build_weights()
_MODULE = None


def _build_module() -> bass.Bass:
    nc = bacc.Bacc("TRN2", target_bir_lowering=False, debug=False)
    x_in = nc.declare_dram_parameter("x", [NG, 128, GRP * 4 * W], F16,
                                     isOutput=False)
    w_in = nc.declare_dram_parameter("w", [128, 4 * 2 * OUT], F16, isOutput=False)
    # y[g, p, ((ig*2 + khc)*2 + f)*516 + 2*kw + gg] = O_{f+2gg}[128*khc + p, kw]
    y_main = nc.declare_dram_parameter("y", [NG, 128, GRP * 4 * 516], F16,
                                       isOutput=True)
    # yr[j*4 + f*2 + ig, g*516 + 2*kw + gg] = O_{f+2gg}[256 + j, kw]
    y_rem = nc.declare_dram_parameter("yr", [8, NG * 516], F16,
                                     isOutput=True)

    with TileContext(nc) as tc:
        with (
            tc.tile_pool(name="wpool", bufs=1) as wpool,
            tc.tile_pool(name="xpool", bufs=3) as xpool,
            tc.tile_pool(name="ypool", bufs=2) as ypool,
            tc.tile_pool(name="spool", bufs=2) as spool,
            tc.tile_pool(name="rpool", bufs=1) as rpool,
            tc.tile_pool(name="psum", bufs=2, space="PSUM") as pspool,
        ):
            Wt = wpool.tile([128, 4 * 2 * OUT], F16)
            Crem = rpool.tile([8, NG * 516], F16)

            def load_x(g, split=False):
                X = xpool.tile([128, GRP * 4 * W], F16, tag="X", name=f"X_{g}")
                ring2 = nc.gpsimd if split else nc.sync
                nc.sync.dma_start(out=X[:, 0:4 * W], in_=x_in[g, :, 0:4 * W])
                ring2.dma_start(out=X[:, 4 * W:8 * W],
                                in_=x_in[g, :, 4 * W:8 * W])
                return X

            nc.sync.dma_start(out=Wt[:], in_=w_in[:])
            X0 = load_x(0, split=True)
            Wr = Wt[:]

            # Tiny PE op consuming the weight DMA so later matmuls depend
            # on it via PE program order.
            warm = pspool.tile([1, 256], F32, tag="pBB", bufs=2)
            nc.tensor.matmul(warm[:, 0:1], lhsT=Wr[:, 0:1], rhs=Wr[:, 0:1],
                             start=True, stop=True)

            def copy(dst, src, eng):
                if eng == "s":
                    nc.scalar.copy(out=dst, in_=src)
                else:
                    nc.vector.tensor_copy(out=dst, in_=src)

            def chain(lhsT_fn, A, B, ha, hb):
                """One banded, filter-interleaved accumulation chain into
                half `ha` of A-pair tile A (512-elem halves = bank-aligned)
                and half `hb` of B-pair tile B (128-elem halves)."""
                if INTERP_SAFE:
                    segs = [(0, 0, 66, 0, True, False),
                            (1, 64, 66, 0, False, False),
                            (1, 66, 130, 0, False, False),
                            (2, 128, 130, 0, False, False),
                            (2, 130, 194, 0, False, False),
                            (3, 192, 194, 0, False, True),
                            (3, 194, 258, 1, True, True)]
                else:
                    segs = [(0, 0, 66, 0, True, False),
                            (1, 64, 130, 0, False, False),
                            (2, 128, 194, 0, False, False),
                            (3, 192, 194, 0, False, True),
                            (3, 194, 258, 1, True, True)]
                for q, lo, hi, t, st, sp in segs:
                    if t == 0:
                        out = A[:, ha * 512 + 2 * lo:ha * 512 + 2 * hi]
                    else:
                        out = B[:, hb * 128 + 2 * (lo - BSP):
                                hb * 128 + 2 * (hi - BSP)]
                    rhs = Wr[:, q * 2 * OUT + 2 * lo:q * 2 * OUT + 2 * hi]
                    nc.tensor.matmul(out, lhsT=lhsT_fn(q), rhs=rhs,
                                     start=st, stop=sp)

            def pair_views(A, B):
                Ah = A[:].rearrange("p (h k) -> p h k", h=2)[:, :, 0:2 * BSP]
                Bh = B[:].rearrange("p (h k) -> p h k", h=2)
                return Ah, Bh

            def pass1(Xv, Yt, ig):
                """4 paired chains into the group Yt tile:
                Yt[p, cc*1032 + (2*kh + f)*2 + ig]."""
                Ytv = Yt[:].rearrange("p (cc k i) -> p cc k i", cc=4, i=GRP)
                for cp in range(2):  # cc pairs (0,1), (2,3)
                    A = pspool.tile([128, 1024], F32, tag="pAA", bufs=3)
                    B = pspool.tile([128, 256], F32, tag="pBB", bufs=2)
                    for h in range(2):
                        cc = cp * 2 + h
                        chain(lambda q: Xv[:, ig, q, cc * 128:(cc + 1) * 128],
                              A[:], B[:], h, h)
                    Ah, Bh = pair_views(A, B)
                    copy(Ytv[:, 2 * cp:2 * cp + 2, 0:2 * BSP, ig], Ah,
                         "s" if cp == 0 else "v")
                    copy(Ytv[:, 2 * cp:2 * cp + 2, 2 * BSP:516, ig], Bh,
                         "v" if cp == 0 else "s")

            def pass2(Yt, STG, ig, i):
                Ytr = Yt[:]
                Ytv4 = Ytr.rearrange("p (cc k f i) -> p cc k f i",
                                     cc=4, f=2, i=GRP)
                Sv = STG[:].rearrange("p (blk k) -> p blk k", k=516)
                for khc in range(2):  # pair over f
                    A = pspool.tile([128, 1024], F32, tag="pAA", bufs=3)
                    B = pspool.tile([128, 256], F32, tag="pBB", bufs=2)
                    for f in range(2):
                        chain(lambda q: Ytv4[:, q,
                                             128 * khc:128 * (khc + 1), f, ig],
                              A[:], B[:], f, f)
                    Ah, Bh = pair_views(A, B)
                    base = (ig * 2 + khc) * 2
                    copy(Sv[:, base:base + 2, 0:2 * BSP], Ah,
                         "s" if khc == 0 else "v")
                    copy(Sv[:, base:base + 2, 2 * BSP:516], Bh,
                         "v" if khc == 0 else "s")

            def rem(Yt, g):
                # remainder rows kh in {256,257} for the whole group: lhsT =
                # 8 contiguous tail cols of each Yt block (order (j, f, ig));
                # psum rows j*4 + f*2 + ig.  One pAA tile: A part in bank 0,
                # B part at the start of bank 1.
                Ytr = Yt[:]
                Rt = pspool.tile([8, 1024], F32, tag="pAA", bufs=3)
                chain(lambda q: Ytr[:, q * 1032 + 1024:(q + 1) * 1032],
                      Rt[:], Rt[:], 0, 4)
                copy(Crem[:, g * 516:g * 516 + 2 * BSP], Rt[:, 0:2 * BSP], "v")
                copy(Crem[:, g * 516 + 2 * BSP:(g + 1) * 516],
                     Rt[:, 512:640], "s")

            # software pipeline: PE runs pass1(i+1) before pass2(i)
            Xg = {0: X0, 1: load_x(1)}
            Xv = {g: Xg[g][:].rearrange("p (i q c) -> p i q c", i=GRP, q=4)
                  for g in (0, 1)}
            Ytg = {0: ypool.tile([128, 4 * 516 * GRP], F16, tag="Yt",
                     name="Yt_0")}
            pass1(Xv[0], Ytg[0], 0)
            STG = None
            for i in range(IMGS):
                g, ig = divmod(i, GRP)
                if ig == 0:
                    if g + 2 < NG:
                        Xg[g + 2] = load_x(g + 2)
                        Xv[g + 2] = Xg[g + 2][:].rearrange(
                            "p (i q c) -> p i q c", i=GRP, q=4)
                    STG = spool.tile([128, GRP * 4 * 516], F16, tag="STG")
                if i + 1 < IMGS:
                    g1, ig1 = divmod(i + 1, GRP)
                    if g1 not in Ytg:
                        Ytg[g1] = ypool.tile([128, 4 * 516 * GRP], F16,
                                             tag="Yt", name=f"Yt_{g1}")
                    pass1(Xv[g1], Ytg[g1], ig1)
                pass2(Ytg[g], STG, ig, i)
                if ig == 0:
                    nc.gpsimd.dma_start(out=y_main[g, :, 0:4 * 516],
                                        in_=STG[:, 0:4 * 516])
                if ig == GRP - 1:
                    rem(Ytg[g], g)
                    del Ytg[g]
                    ring = nc.sync if g == NG - 1 else nc.gpsimd
                    ring.dma_start(out=y_main[g, :, 4 * 516:8 * 516],
                                   in_=STG[:, 4 * 516:8 * 516])
                if i % 8 == 7:
                    c = i // 8
                    nc.gpsimd.dma_start(
                        out=y_rem[:, c * 4 * 516:(c + 1) * 4 * 516],
                        in_=Crem[:, c * 4 * 516:(c + 1) * 4 * 516])
    nc.finalize()
    return nc


def _get_module() -> bass.Bass:
    global _MODULE
    if _MODULE is None:
        _MODULE = _build_module()
    return _MODULE


def make_in_maps(x: np.ndarray) -> list[dict]:
    x = np.asarray(x, dtype=np.float32)
    B, C, Hx, Wx = x.shape
    assert (Hx, Wx) == (H, W) and B * C == N_CORES * IMGS
    imgs = x.reshape(B * C, H, W)
    maps = []
    for k in range(N_CORES):
        # X[g][p, ig*2048 + q*512 + c] = x[g*GRP+ig, 128q + p, c]
        xc = imgs[k * IMGS:(k + 1) * IMGS].reshape(NG, GRP, 4, 128, W)
        xc = np.ascontiguousarray(xc.transpose(0, 3, 1, 2, 4))
        maps.append({"x": xc.reshape(NG, 128, GRP * 4 * W).astype(np.float16),
                     "w": _WEIGHTS})
    return maps


def kernel(**inputs) -> np.ndarray:
    x = np.asarray(inputs["x"], dtype=np.float32)
    B, C, Hx, Wx = x.shape

    nc = _get_module()
    in_maps = make_in_maps(x)
    res = run_bass_kernel_spmd(nc, in_maps, list(range(N_CORES))).results

    full = np.empty((N_CORES * IMGS, 4, OUT, OUT), dtype=np.float32)
    for k in range(N_CORES):
        # [g, p, ig, khc, f, kw, gg]
        ym = res[k]["y"].reshape(NG, 128, GRP, 2, 2, OUT, 2)
        yr = res[k]["yr"].reshape(8, NG, OUT, 2)  # [j*4+f*2+ig, g, kw, gg]
        dst = full[k * IMGS:(k + 1) * IMGS]
        # dst[g*GRP+ig, f+2gg, khc*128+p, kw] = ym[g, p, ig, khc, f, kw, gg]
        t = ym.transpose(0, 2, 4, 6, 3, 1, 5).reshape(IMGS, 4, 256, OUT)
        # t's dim-1 is f*2+gg; reorder to s = f+2gg -> fg indices [0,2,1,3]
        dst[:, :, :256, :] = t[:, [0, 2, 1, 3]]
        for f in range(2):
            for j in range(2):
                for g in range(2):
                    for ig in range(GRP):
                        dst[ig::GRP, f + 2 * g, 256 + j, :] = \
                            yr[j * 4 + f * 2 + ig, :, :, g]

    return np.ascontiguousarray(full.reshape(B, 4 * C, OUT, OUT))

